# revision 1
# baseline (speedup 1.0000x reference)
"""Trainium2 Bass kernel for nn_BaseAggregator_31439160607279.

Math (reference):
  af (a,c,f,t), imf (v,c,h,w), split c into k=2 heads of 256 ch.
  sims[a,v,k,hw,t] = sum_c af*imf ; + cls[a,v,k] ; relu ; max over hw ;
  masked mean over t (mask m[a,t] in {0,1}, den = f*sum_t m) ; sum over k.

Strategy:
  - Shard the image dim v=32 across 8 cores (4 images/core); audio replicated.
  - Pack ALL mask-active (a, t) pairs (all 32 audios) into the matmul M dim
    (m=0 columns contribute nothing to the masked sum) -> ~3219 rows -> 26
    M-tiles of 128 with ~3% padding.
  - Big matmuls in fp16 (PE upcasts to FP22; 1 cycle/row streaming):
      lhsT = packed audio rows, channel-chunk (K=128, M=128)
      rhs  = [imf[v0,k] | imf[v1,k]] (K=128, N=392), accumulate 2 chunks;
      two image pairs (all 4 local images) share one 2-bank PSUM tile.
  - relu(max_hw(x)+cls) == max_hw(relu(x+cls)): reduce_max on raw PSUM
    (one DVE op per 2-bank PSUM group), then add row-broadcast cls (one-hot
    matmul), relu on the Scalar engine.
  - Masked t-sum via matmul with one-hot audio columns (K=packed rows,
    M=32 audios), accumulated across M-tiles in a single PSUM bank;
  - divide by den, sum heads; host concatenates core outputs along v.
"""

import math
from contextlib import ExitStack

import numpy as np

import concourse.bacc as bacc
import concourse.mybir as mybir
import concourse.tile as tile
from concourse.bass_utils import run_bass_kernel_spmd

# Problem dims (hardcoded per spec)
A, V, C, F, T, H, W = 32, 32, 512, 1, 200, 14, 14
K = 2                    # heads
NCH = C // K             # 256 channels per head
KC = 2                   # channel chunks per head
KP = NCH // KC           # 128 = contraction per matmul
HW = H * W               # 196
NCORES = 8
VL = V // NCORES         # 4 local images per core
NVP = VL // 2            # 2 local image pairs
NPAIR = 2 * HW           # 392 = matmul free dim per image pair

GATHER = True            # pack only mask-active (a, t) rows
AFP_CHUNK = 7            # M-tiles per audio DMA chunk

TRACE = False
LAST_RESULTS = None

_kernel_cache = {}

f32 = mybir.dt.float32
f16 = mybir.dt.float16
X = mybir.AxisListType.X


def _build(MT: int):
    """Build + compile the per-core Bass program for MT packed-row tiles."""
    nc = bacc.Bacc("TRN2", target_bir_lowering=False, debug=False)

    afp_d = nc.dram_tensor("afp", (K, KC, KP, MT * 128), f16, kind="ExternalInput")
    imf_d = nc.dram_tensor("imf", (KP, K * KC * VL * HW), f16, kind="ExternalInput")
    # aux = [acls (K*KC*A=128) | icls (K*KC*VL=16) | maskc (MT*A)] along free dim
    aux_d = nc.dram_tensor("aux", (KP, 144 + MT * A), f16, kind="ExternalInput")
    onehot_d = nc.dram_tensor("onehot", (A, MT * 128), f16, kind="ExternalInput")
    maskf_d = nc.dram_tensor("maskf", (A, T), f32, kind="ExternalInput")
    outk_d = nc.dram_tensor("outk", (A, K * VL), f32, kind="ExternalOutput")
    outsum_d = nc.dram_tensor("outsum", (A, VL), f32, kind="ExternalOutput")

    with tile.TileContext(nc) as tc, ExitStack() as ctx:
        cst = ctx.enter_context(tc.tile_pool(name="cst", bufs=1))
        ps_big = ctx.enter_context(tc.tile_pool(name="ps_big", bufs=3, space="PSUM"))
        ps_sm = ctx.enter_context(tc.tile_pool(name="ps_sm", bufs=1, space="PSUM"))
        ps_num = ctx.enter_context(tc.tile_pool(name="ps_num", bufs=1, space="PSUM"))
        sm_pool = ctx.enter_context(tc.tile_pool(name="sm", bufs=3))

        # --- persistent SBUF tiles ---
        afp_sb = {}
        for k in range(K):
            for kc in range(KC):
                afp_sb[k, kc] = cst.tile([KP, MT * 128], f16, tag=f"afp{k}{kc}", name=f"afp{k}{kc}")
        imf_sb = cst.tile([KP, K * KC * VL * HW], f16, tag="imf", name="imf_sb")
        aux_sb = cst.tile([KP, 144 + MT * A], f16, tag="aux", name="aux_sb")
        onehot_sb = cst.tile([A, MT * 128], f16, tag="onehot", name="onehot_sb")
        maskf_sb = cst.tile([A, T], f32, tag="maskf", name="maskf_sb")

        def imf_rhs(k, kc, sub):
            off = (k * KC + kc) * (VL * HW) + sub * NPAIR
            return imf_sb[:, off:off + NPAIR]

        def acls_lhs(k, kc):
            off = (k * KC + kc) * A
            return aux_sb[:, off:off + A]

        def icls_rhs(k, kc):
            off = 128 + (k * KC + kc) * VL
            return aux_sb[:, off:off + VL]

        def maskc_lhs(mt):
            off = 144 + mt * A
            return aux_sb[:, off:off + A]

        # DMA order: compute-gating transfers first, all with fat descriptors.
        half = K * KC * VL * HW // 2
        nc.sync.dma_start(out=imf_sb[:, 0:half], in_=imf_d.ap()[:, 0:half])
        afp_cuts = [0, min(2, MT), min(6, MT)] + [min(MT, c) for c in range(AFP_CHUNK * 2, MT + AFP_CHUNK, AFP_CHUNK)]
        afp_cuts = sorted(set(afp_cuts))
        c0 = afp_cuts[1] * 128
        for k in range(K):
            for kc in range(KC):
                nc.sync.dma_start(out=afp_sb[k, kc][:, 0:c0], in_=afp_d.ap()[k, kc][:, 0:c0])
        nc.sync.dma_start(out=imf_sb[:, half:2 * half], in_=imf_d.ap()[:, half:2 * half])
        chunks = list(zip(afp_cuts[1:-1], afp_cuts[2:]))

        def afp_chunk_dma(lo, hi):
            sl = slice(lo * 128, hi * 128)
            for k in range(K):
                for kc in range(KC):
                    nc.sync.dma_start(out=afp_sb[k, kc][:, sl], in_=afp_d.ap()[k, kc][:, sl])

        if chunks:
            afp_chunk_dma(*chunks[0])
        nc.sync.dma_start(out=aux_sb[:], in_=aux_d.ap())
        nc.sync.dma_start(out=onehot_sb[:], in_=onehot_d.ap())
        nc.sync.dma_start(out=maskf_sb[:], in_=maskf_d.ap())
        for lo, hi in chunks[1:]:
            afp_chunk_dma(lo, hi)

        # --- PE warm-up: keep the PE busy during the input DMA so the HAM
        # clock-gate reaches 8/8 before the real matmuls arrive ---
        warm = cst.tile([KP, 512], f16, tag="warm", name="warm_sb")
        nc.vector.memset(warm[:], 0.0)
        for w in range(8):
            pw = ps_big.tile([128, 1024], f32, tag="ps_big", name="ps_warm")
            nc.tensor.matmul(pw[:, 0:512], lhsT=warm[:, 0:128], rhs=warm[:],
                             start=True, stop=True)

        # --- main loop over M-tiles (cls_sims emitted after mt0's sims MMs) ---
        cls_sb = cst.tile([A, K * VL], f16, tag="cls", name="cls_sb")
        clsb_sb = cst.tile([128, MT * K * VL], f16, tag="clsb", name="clsb_sb")
        num_ps = ps_num.tile([A, K * VL], f32, tag="ps_num", name="ps_numacc")
        den = cst.tile([A, 1], f32, tag="den", name="den")
        rden = cst.tile([A, 1], f32, tag="rden", name="rden")
        sm3_tiles = []
        smraw_tiles = []

        def emit_group(mt, k):
            ps = ps_big.tile([128, 1024], f32, tag="ps_big", name="ps_sims")
            for sub in range(NVP):
                for kc in range(KC):
                    nc.tensor.matmul(
                        ps[:, sub * 512:sub * 512 + NPAIR],
                        lhsT=afp_sb[k, kc][:, mt * 128:(mt + 1) * 128],
                        rhs=imf_rhs(k, kc, sub),
                        start=(kc == 0), stop=(kc == 1),
                    )
            rview = ps[:].rearrange("p (b q) -> p b q", b=2)[:, :, 0:NPAIR]
            rview = rview.rearrange("p b (i x) -> p b i x", i=2)
            nc.vector.reduce_max(smraw_tiles[mt][:, k * VL:(k + 1) * VL], rview, axis=X)

        for mt in range(MT):
            smraw = sm_pool.tile([128, K * VL], f16, tag="smraw", name="smraw", bufs=5)
            smraw_tiles.append(smraw)
            # k1 groups run one M-tile behind k0 so mt0-k1 never waits on the
            # second half of the image DMA
            emit_group(mt, 0)
            if mt >= 1:
                emit_group(mt - 1, 1)
            if mt == MT - 1:
                emit_group(mt, 1)

            if mt == min(2, MT - 1):
                for k in range(K):
                    pc = ps_sm.tile([A, VL], f32, tag="ps_sm", name="ps_cls")
                    for kc in range(KC):
                        nc.tensor.matmul(
                            pc[:], lhsT=acls_lhs(k, kc), rhs=icls_rhs(k, kc),
                            start=(kc == 0), stop=(kc == 1),
                        )
                    nc.vector.tensor_copy(cls_sb[:, k * VL:(k + 1) * VL], pc[:])
            if mt == min(3, MT - 1):
                nc.vector.reduce_sum(den[:], maskf_sb[:], axis=X)
                nc.vector.reciprocal(rden[:], den[:])
            # spread the cls row-broadcasts over iterations, two behind
            bt_lo = max(0, 2 * (mt - 2))
            bt_hi = min(2 * (mt - 1), MT) if mt >= 2 else 0
            if mt == MT - 1:
                bt_hi = MT
            for bt in range(bt_lo, bt_hi):
                pb = ps_sm.tile([128, K * VL], f32, tag="ps_sm", name="ps_bcast")
                nc.tensor.matmul(pb[:], lhsT=onehot_sb[:, bt * 128:(bt + 1) * 128],
                                 rhs=cls_sb[:], start=True, stop=True)
                nc.scalar.copy(clsb_sb[:, bt * K * VL:(bt + 1) * K * VL], pb[:])

            def emit_addrelu(j):
                sm2 = sm_pool.tile([128, K * VL], f16, tag="sm2", name="sm2")
                nc.gpsimd.tensor_add(sm2[:], smraw_tiles[j][:],
                                     clsb_sb[:, j * K * VL:(j + 1) * K * VL])
                sm3 = sm_pool.tile([128, K * VL], f16, tag="sm3", name="sm3", bufs=8)
                nc.scalar.activation(sm3[:], sm2[:], mybir.ActivationFunctionType.Relu)
                sm3_tiles.append(sm3)

            def emit_numdot(j):
                nc.tensor.matmul(num_ps[:], lhsT=maskc_lhs(j), rhs=sm3_tiles[j][:],
                                 start=(j == 0), stop=(j == MT - 1))

            # chains delayed so no PE-queue instruction waits on a fresh
            # cross-engine dependency: add/relu for mt-1, masked-sum for mt-3
            if mt >= 2:
                emit_addrelu(mt - 2)
            if mt >= 5:
                emit_numdot(mt - 5)
        for j in range(max(MT - 2, 0), MT):
            emit_addrelu(j)
        for j in range(max(MT - 5, 0), MT):
            emit_numdot(j)

        # --- divide, head-sum, out ---
        outk_sb = cst.tile([A, K * VL], f32, tag="outk", name="outk_sb")
        nc.vector.tensor_scalar_mul(outk_sb[:], num_ps[:], rden[:])
        outsum_sb = cst.tile([A, VL], f32, tag="outsum", name="outsum_sb")
        nc.vector.tensor_add(outsum_sb[:], outk_sb[:, 0:VL], outk_sb[:, VL:2 * VL])
        nc.sync.dma_start(out=outk_d.ap(), in_=outk_sb[:])
        nc.sync.dma_start(out=outsum_d.ap(), in_=outsum_sb[:])

    nc.compile()
    return nc


def prepare_inputs(audio_feats, image_feats, audio_cls, image_cls, audio_mask):
    """Host-side shard + layout prep. Returns (MT, in_maps)."""
    af = np.ascontiguousarray(audio_feats, dtype=np.float32).reshape(A, K, KC, KP, T)
    imf = np.ascontiguousarray(image_feats, dtype=np.float32).reshape(V, K, KC, KP, HW)
    acls = np.ascontiguousarray(audio_cls, dtype=np.float32).reshape(A, K, KC, KP)
    icls = np.ascontiguousarray(image_cls, dtype=np.float32).reshape(V, K, KC, KP)
    mask = np.asarray(audio_mask)
    maskf = np.ascontiguousarray(mask.astype(np.float32))

    if GATHER:
        rows_a, rows_t = np.nonzero(mask != 0)
        mvals = np.ones(len(rows_a), np.float32)
    else:
        rows_a, rows_t = np.indices((A, T)).reshape(2, -1)
        mvals = maskf[rows_a, rows_t]
    L = len(rows_a)
    MT = max(1, math.ceil(L / 128))
    LP = MT * 128

    # audio rows, shared by all cores: (K, KC, KP, MT*128) fp16
    af_rows = np.zeros((LP, K, KC, KP), np.float32)
    af_rows[:L] = af[rows_a, :, :, :, rows_t]
    afp = np.ascontiguousarray(
        af_rows.transpose(1, 2, 3, 0).reshape(K, KC, KP, MT * 128)
    ).astype(np.float16)

    oh = np.zeros((LP, A), np.float16)
    oh[np.arange(L), rows_a] = 1.0
    onehot = np.ascontiguousarray(oh.T)                       # (A, MT*128)
    mc = np.zeros((LP, A), np.float16)
    mc[np.arange(L), rows_a] = mvals
    maskc = mc.reshape(MT, 128, A).transpose(1, 0, 2).reshape(128, MT * A)
    acls_h = acls.transpose(3, 1, 2, 0).reshape(KP, K * KC * A)

    in_maps = []
    for ci in range(NCORES):
        vsl = slice(ci * VL, (ci + 1) * VL)
        imf_h = np.ascontiguousarray(
            imf[vsl].transpose(3, 1, 2, 0, 4).reshape(KP, K * KC * VL * HW)
        ).astype(np.float16)
        icls_h = icls[vsl].transpose(3, 1, 2, 0).reshape(KP, K * KC * VL)
        aux = np.concatenate(
            [acls_h, icls_h, maskc], axis=1
        ).astype(np.float16)
        in_maps.append({
            "afp": afp,
            "imf": imf_h,
            "aux": np.ascontiguousarray(aux),
            "onehot": onehot,
            "maskf": maskf,
        })
    return MT, in_maps


def get_program(MT: int):
    if MT not in _kernel_cache:
        _kernel_cache[MT] = _build(MT)
    return _kernel_cache[MT]


def kernel(audio_feats, image_feats, audio_cls, image_cls, audio_mask, agg_heads):
    global LAST_RESULTS
    MT, in_maps = prepare_inputs(
        audio_feats, image_feats, audio_cls, image_cls, audio_mask
    )
    nc = get_program(MT)
    res = run_bass_kernel_spmd(nc, in_maps, list(range(NCORES)), trace=TRACE)
    LAST_RESULTS = res
    agg = bool(np.asarray(agg_heads))
    outs = []
    for ci in range(NCORES):
        if agg:
            outs.append(res.results[ci]["outsum"])  # (A, VL)
        else:
            outk = res.results[ci]["outk"].reshape(A, K, VL)
            outs.append(outk.transpose(0, 2, 1))    # (A, VL, K)
    return np.concatenate(outs, axis=1).astype(np.float32)



# revision 2
# speedup vs baseline: 1.2798x; 1.2798x over previous
"""Trainium2 Bass kernel for nn_BaseAggregator_31439160607279 (v2).

Math (reference):
  af (a,c,f,t), imf (v,c,h,w), split c into k=2 heads of 256 ch.
  sims[a,v,k,hw,t] = sum_c af*imf ; + cls[a,v,k] ; relu ; max over hw ;
  masked mean over t (mask m[a,t] in {0,1}, den = f*sum_t m) ; sum over k.

Strategy v2 (measured-rate driven):
  - 2D shard: RS=2 row-shards x VS=4 image-shards (VL=8 images/core).
  - Mask-active (a,t) rows packed; MTloc 128-row tiles per core.
  - fp8e4 DoubleRow matmuls: one 392-col matmul covers an image pair with
    the full 256-ch head contraction (~199ns/matmul measured).
  - PSUM: 2 rotating 4-bank tiles, 2 groups (4 images) per tile.
  - Consumers (the bottleneck, ~1.5ns/PSUM col/engine):
      per mt: group g0 -> DVE direct reduce_max;
              groups g1-g3 -> Act copies to fp16 SBUF (784/1568-col ops),
              then per-2mt batched DVE tensor_tensor max-fold chains
              (tt fp16 ~0.53ns/out col) + one small reduce.
  - cls_sims/rden host-precomputed into clsb/mkd tables; per-mt masked
    t-sum matmuls accumulate in PSUM after the loop (tail).
  - Host: sum row-shard partials, concat image shards, sum heads.
"""

import math
from contextlib import ExitStack

import numpy as np
import ml_dtypes

import concourse.bacc as bacc
import concourse.mybir as mybir
import concourse.tile as tile
from concourse.bass_utils import run_bass_kernel_spmd

A, V, C, F, T, H, W = 32, 32, 512, 1, 200, 14, 14
K = 2
KP = 128
HW = H * W               # 196
NCORES = 8

RS, VS = 2, 4
VL = V // VS             # 8 images per core
NQ = VL // 4             # 2 quads per (mt, k)
NG = K * NQ              # 4 groups per mt

TRACE = False
LAST_RESULTS = None
_kernel_cache = {}

f32 = mybir.dt.float32
f16 = mybir.dt.float16
f8 = mybir.dt.float8e4
X = mybir.AxisListType.X
DR = mybir.MatmulPerfMode.DoubleRow
MX = mybir.AluOpType.max
NDUMMY = 0


def _build(MTloc: int):
    nc = bacc.Bacc("TRN2", target_bir_lowering=False, debug=False)

    afp_d = nc.dram_tensor("afp", (K, MTloc, KP, 256), f8, kind="ExternalInput")
    imf_d = nc.dram_tensor("imf", (K, NQ, KP, 1568), f8, kind="ExternalInput")
    clsb_d = nc.dram_tensor("clsb", (KP, MTloc * 16), f16, kind="ExternalInput")
    mkd_d = nc.dram_tensor("mkd", (KP, MTloc * A), f16, kind="ExternalInput")
    outk_d = nc.dram_tensor("outk", (A, 16), f32, kind="ExternalOutput")

    with tile.TileContext(nc) as tc, ExitStack() as ctx:
        cst = ctx.enter_context(tc.tile_pool(name="cst", bufs=1))
        ps = ctx.enter_context(tc.tile_pool(name="ps", bufs=2, space="PSUM"))
        acp_pool = ctx.enter_context(tc.tile_pool(name="acp", bufs=2))
        fld = ctx.enter_context(tc.tile_pool(name="fld", bufs=2))

        afp_sb = [cst.tile([KP, MTloc * 256], f8, tag=f"afp{k}", name=f"afp{k}")
                  for k in range(K)]
        imf_sb = [cst.tile([KP, NQ * 1568], f8, tag=f"imf{k}", name=f"imf{k}")
                  for k in range(K)]
        clsb_sb = cst.tile([KP, MTloc * 16], f16, tag="clsb", name="clsb")
        mkd_sb = cst.tile([KP, MTloc * A], f16, tag="mkd", name="mkd")
        smraw = cst.tile([KP, MTloc * 16], f16, tag="smraw", name="smraw")
        sm3 = cst.tile([KP, MTloc * 16], f16, tag="sm3", name="sm3")

        def afp_dma(k, lo, hi):
            for t in range(lo, hi):
                nc.sync.dma_start(
                    out=afp_sb[k][:, t * 256:(t + 1) * 256],
                    in_=afp_d.ap()[k, t])

        nc.sync.dma_start(out=imf_sb[0][:, 0:1568], in_=imf_d.ap()[0, 0])
        nc.sync.dma_start(out=imf_sb[0][:, 1568:3136], in_=imf_d.ap()[0, 1])
        afp_dma(0, 0, min(2, MTloc))
        nc.sync.dma_start(out=imf_sb[1][:, 0:1568], in_=imf_d.ap()[1, 0])
        nc.sync.dma_start(out=imf_sb[1][:, 1568:3136], in_=imf_d.ap()[1, 1])
        afp_dma(1, 0, min(2, MTloc))
        nc.sync.dma_start(out=clsb_sb[:], in_=clsb_d.ap())
        nc.sync.dma_start(out=mkd_sb[:], in_=mkd_d.ap())
        for lo in range(2, MTloc, 4):
            hi = min(lo + 4, MTloc)
            afp_dma(0, lo, hi)
            afp_dma(1, lo, hi)

        # PE warm-up during DMA + fp8 dummy operands for p-state keepalive
        warm = cst.tile([KP, 512], f16, tag="warm", name="warm")
        nc.vector.memset(warm[:], 0.0)
        warm8 = cst.tile([KP, 256 + 240], f8, tag="warm8", name="warm8")
        nc.vector.memset(warm8[:], 0.0)
        w8stat = warm8[:, 0:256].rearrange("p (i m) -> p i m", i=2)
        w8rhs = warm8[:, 256:496].rearrange("p (i n) -> p i n", i=2)
        for _ in range(8):
            pw = ps.tile([128, 2048], f32, tag="ps", name="pw")
            nc.tensor.matmul(pw[:, 0:512], lhsT=warm[:, 0:128], rhs=warm[:],
                             start=True, stop=True)

        # ---- main loop: per mt emit 2 PSUM tiles (head 0, head 1) ----
        # tile layout: group q=0 at cols 0:392 & 512:904,
        #              group q=1 at cols 1024:1416 & 1536:1928
        def emit_tile(mt, k):
            stat = afp_sb[k][:, mt * 256:(mt + 1) * 256].rearrange(
                "p (i m) -> p i m", i=2)
            pst = ps.tile([128, 2048], f32, tag="ps", name=f"t{mt}_{k}")
            for q in range(NQ):
                rview = imf_sb[k][:, q * 1568:(q + 1) * 1568].rearrange(
                    "p (i n) -> p i n", i=2)
                off = q * 1024
                nc.tensor.matmul(pst[:, off:off + 392], lhsT=stat,
                                 rhs=rview[:, :, 0:392],
                                 start=True, stop=True, perf_mode=DR)
                nc.tensor.matmul(pst[:, off + 512:off + 904], lhsT=stat,
                                 rhs=rview[:, :, 392:784],
                                 start=True, stop=True, perf_mode=DR)
            # p-state keepalive: dummy DR matmuls into spare tile columns
            for dq in range(NDUMMY):
                doff = (dq % 4) * 512 + 392
                nc.tensor.matmul(pst[:, doff:doff + 120], lhsT=w8stat,
                                 rhs=w8rhs, start=True, stop=True,
                                 perf_mode=DR)
            return pst

        def tile_view(pst, qlo, nq):
            # [p, nq*2, 2, 196] over groups qlo..qlo+nq of this tile
            v = pst[:].rearrange("p (s q2) -> p s q2", s=4)[
                :, 2 * qlo:2 * (qlo + nq), 0:392]
            return v.rearrange("p s (i x) -> p s i x", i=2)

        def emit_chain(emt, acp):
            nblk = 6 if emt + 1 < MTloc or MTloc % 2 == 0 else 3
            nim = nblk * 4
            src = acp[:, 0:nblk * 784].rearrange("p (n x) -> p n x", n=nim)
            sizes = [(196, 98), (98, 49), (49, 25), (25, 13), (13, 7), (7, 4)]
            cur = src
            for (w, wo) in sizes:
                dst = fld.tile([KP, 24 * wo], f16, tag=f"f{w}",
                               name=f"f{w}_{emt}", bufs=2)
                dv = dst[:, 0:nim * wo].rearrange("p (n x) -> p n x", n=nim)
                nc.vector.tensor_tensor(
                    dv, cur[:, :, 0:wo], cur[:, :, w - wo:w], MX)
                cur = dv
            # reduce [p, nim, 13] -> [p, nim] into smraw slots
            if nblk == 6:
                ov = smraw[:].rearrange("p (m c) -> p m c", m=MTloc)[
                    :, emt:emt + 2, 4:16]
            else:
                ov = smraw[:, emt * 16 + 4:emt * 16 + 16]
            nc.vector.reduce_max(ov, cur, axis=X)
            lo = emt * 16
            hi = (emt + nblk // 3) * 16
            nc.gpsimd.tensor_add(sm3[:, lo:hi], smraw[:, lo:hi],
                                 clsb_sb[:, lo:hi])
            nc.vector.tensor_scalar_max(sm3[:, lo:hi], sm3[:, lo:hi], 0.0)

        pending_pairs = []
        acp_tiles = {}
        for mt in range(MTloc):
            # head 0 tile: g0 (q0) = DVE direct, g1 (q1) = Act copy
            if mt % 2 == 0:
                acp = acp_pool.tile([KP, 6 * 784], f16, tag="acp",
                                    name=f"acp{mt}")
                acp_tiles[mt] = acp
            else:
                acp = acp_tiles[mt - 1]
            half = (mt % 2) * 3 * 784

            pstA = emit_tile(mt, 0)
            base = mt * 16
            nc.vector.reduce_max(
                smraw[:, base:base + 4].rearrange("p (s i) -> p s i", s=2),
                tile_view(pstA, 0, 1), axis=X)
            nc.scalar.copy(
                acp[:, half:half + 784].rearrange(
                    "p (s i x) -> p s i x", s=2, i=2),
                tile_view(pstA, 1, 1))

            pstB = emit_tile(mt, 1)
            nc.scalar.copy(
                acp[:, half + 784:half + 2352].rearrange(
                    "p (s i x) -> p s i x", s=4, i=2),
                tile_view(pstB, 0, 2))

            if mt % 2 == 1:
                emit_chain(mt - 1, acp)
            elif mt == MTloc - 1:
                emit_chain(mt, acp)

        # ---- tail: masked t-sum accumulation + out ----
        pnum = ps.tile([128, 2048], f32, tag="ps", name="pnum")
        for mt in range(MTloc):
            nc.tensor.matmul(pnum[0:A, 0:16],
                             lhsT=mkd_sb[:, mt * A:(mt + 1) * A],
                             rhs=sm3[:, mt * 16:(mt + 1) * 16],
                             start=(mt == 0), stop=(mt == MTloc - 1))
        outk_sb = cst.tile([A, 16], f32, tag="outk", name="outk")
        nc.scalar.copy(outk_sb[:], pnum[0:A, 0:16])
        nc.sync.dma_start(out=outk_d.ap(), in_=outk_sb[:])

    nc.compile()
    return nc


def prepare_inputs(audio_feats, image_feats, audio_cls, image_cls, audio_mask):
    af5 = np.ascontiguousarray(audio_feats, np.float32).reshape(A, K, 2, KP, T)
    imf5 = np.ascontiguousarray(image_feats, np.float32).reshape(V, K, 2, KP, HW)
    maskb = np.asarray(audio_mask) != 0
    rows_a, rows_t = np.nonzero(maskb)
    L = len(rows_a)
    MTtot = max(1, math.ceil(L / 128))
    MTloc = max(1, math.ceil(MTtot / RS))
    cap = RS * MTloc * 128

    af_rows = np.zeros((cap, K, 2, KP), np.float32)
    af_rows[:L] = af5[rows_a, :, :, :, rows_t]
    a_of_row = np.full(cap, -1, np.int64)
    a_of_row[:L] = rows_a

    cls_full = np.einsum(
        "akc,vkc->avk",
        np.asarray(audio_cls, np.float32).reshape(A, K, C // K),
        np.asarray(image_cls, np.float32).reshape(V, K, C // K),
    ).astype(np.float32)
    rden = 1.0 / (F * maskb.sum(1).astype(np.float32))

    imf8_all = []
    for vs in range(VS):
        arr = np.zeros((K, NQ, KP, 2, 2, 2, HW), np.float32)
        for q in range(NQ):
            for j2 in range(2):
                for im in range(2):
                    v = vs * VL + q * 4 + j2 * 2 + im
                    arr[:, q, :, :, j2, im, :] = imf5[v].transpose(0, 2, 1, 3)
        imf8_all.append(np.ascontiguousarray(
            arr.reshape(K, NQ, KP, 1568)).astype(ml_dtypes.float8_e4m3))

    in_maps = []
    for rs in range(RS):
        sl = slice(rs * MTloc * 128, (rs + 1) * MTloc * 128)
        chunk = af_rows[sl]
        a_chunk = a_of_row[sl]
        afp = np.ascontiguousarray(
            chunk.reshape(MTloc, 128, K, 2, KP).transpose(2, 0, 4, 3, 1)
            .reshape(K, MTloc, KP, 256)).astype(ml_dtypes.float8_e4m3)

        mkd = np.zeros((MTloc, 128, A), np.float32)
        rr = np.arange(MTloc * 128)
        valid = a_chunk >= 0
        mkd[rr[valid] // 128, rr[valid] % 128, a_chunk[valid]] = \
            rden[a_chunk[valid]]
        mkd = np.ascontiguousarray(
            mkd.transpose(1, 0, 2).reshape(128, MTloc * A)).astype(np.float16)

        for vs in range(VS):
            clsb = np.zeros((MTloc * 128, NG, 4), np.float32)
            for g in range(NG):
                k, q = divmod(g, NQ)
                vbase = vs * VL + q * 4
                cv = cls_full[:, vbase:vbase + 4, k]
                clsb[valid, g, :] = cv[a_chunk[valid]]
            clsb = np.ascontiguousarray(
                clsb.reshape(MTloc, 128, NG * 4).transpose(1, 0, 2)
                .reshape(128, MTloc * NG * 4)).astype(np.float16)
            in_maps.append({
                "afp": afp,
                "imf": imf8_all[vs],
                "clsb": clsb,
                "mkd": mkd,
            })
    return MTloc, in_maps


def get_program(MTloc: int):
    if MTloc not in _kernel_cache:
        _kernel_cache[MTloc] = _build(MTloc)
    return _kernel_cache[MTloc]


def kernel(audio_feats, image_feats, audio_cls, image_cls, audio_mask, agg_heads):
    global LAST_RESULTS
    MTloc, in_maps = prepare_inputs(
        audio_feats, image_feats, audio_cls, image_cls, audio_mask
    )
    nc = get_program(MTloc)
    res = run_bass_kernel_spmd(nc, in_maps, list(range(NCORES)), trace=TRACE)
    LAST_RESULTS = res
    agg = bool(np.asarray(agg_heads))
    outk = np.zeros((A, V, K), np.float32)
    for rs in range(RS):
        for vs in range(VS):
            o = np.asarray(res.results[rs * VS + vs]["outk"], np.float32)
            o = o.reshape(A, NG, 4)
            for g in range(NG):
                k, q = divmod(g, NQ)
                vbase = vs * VL + q * 4
                outk[:, vbase:vbase + 4, k] += o[:, g, :]
    if agg:
        return outk.sum(2).astype(np.float32)
    return outk.astype(np.float32)
